# revision 1
# baseline (speedup 1.0000x reference)
"""CenterLoss Trainium2 kernel.

loss = (sum_i clamp(||x_i - centers[labels_i]||^2, 1e-12, 1e12)
        + BS*(C_OUT-1)*1e-12) / BS

Masking the full [BS, C_OUT] distance matrix keeps exactly one distance
per row; the other BS*(C_OUT-1) clamped zeros are a host-side constant.

Data-parallel over batch across 8 NeuronCores, centers replicated.  Each
core gathers its 1024 label rows with ONE dma_gather ucode instruction
(Q7 descriptor emission ~8.5ns/desc is the bottleneck; one instruction
avoids 8x instruction overhead of stock indirect DMA).  dma_gather takes
int16 indices (max 32767 < C_OUT=50000), so we gather 512-byte center
PAIRS at idx = label>>1 and select the even/odd half per row from the
label parity on DVE.  The mlp ucode library is preloaded at kernel start
on the otherwise-idle Pool engine.  Raw Bass blocks (no TileContext)
avoid the Tile kernel-tail drain+barrier.  Host sums partials in f64.

Host-side input prep is limited to sharding/replication and index-tensor
layout (int64->int32 and the 16-partition-wrapped x8-replicated index
layout the gather ucode requires); index arithmetic (>>1, &1) and all
data math run on device.
"""

import numpy as np

BS, C_OUT, D = 8192, 50000, 64
N_CORES = 8
ROWS = BS // N_CORES  # rows per core
P = 128  # SBUF partitions
RPP = ROWS // P  # rows per partition (row r lives at [r % P, r // P])
CLAMP_MIN, CLAMP_MAX = 1e-12, 1e12

_CACHE = {}


def _build_program():
    import concourse.bacc as bacc
    import concourse.bass as bass
    import concourse.mybir as mybir
    from concourse.library_config import mlp

    nc = bacc.Bacc(
        "TRN2", target_bir_lowering=False, debug=False, num_devices=N_CORES
    )

    f32 = mybir.dt.float32
    i32 = mybir.dt.int32
    i16 = mybir.dt.int16

    x_d = nc.dram_tensor("x", [ROWS, D], f32, kind="ExternalInput")
    lab_d = nc.dram_tensor("labels", [ROWS], i32, kind="ExternalInput")
    w16_d = nc.dram_tensor("labels_w16", [P, ROWS // 16], i32, kind="ExternalInput")
    cen_d = nc.dram_tensor("centers", [C_OUT, D], f32, kind="ExternalInput")
    out_d = nc.dram_tensor("out", [P, RPP], f32, kind="ExternalOutput")

    from contextlib import ExitStack
    with ExitStack() as ctx:
        x_t = ctx.enter_context(nc.sbuf_tensor("x_t", [P, RPP * D], f32))
        w16_t = ctx.enter_context(nc.sbuf_tensor("w16_t", [P, ROWS // 16], i32))
        shr_t = ctx.enter_context(nc.sbuf_tensor("shr_t", [P, ROWS // 16], i32))
        idx_t = ctx.enter_context(nc.sbuf_tensor("idx_t", [P, ROWS // 16], i16))
        par_i = ctx.enter_context(nc.sbuf_tensor("par_i", [P, RPP], i32))
        parb_i = ctx.enter_context(nc.sbuf_tensor("parb_i", [P, RPP], i32))
        par_f = ctx.enter_context(nc.sbuf_tensor("par_f", [P, RPP], f32))
        pairs_t = ctx.enter_context(nc.sbuf_tensor("pairs_t", [P, RPP * 2 * D], f32))
        de_t = ctx.enter_context(nc.sbuf_tensor("de_t", [P, RPP * D], f32))
        do_t = ctx.enter_context(nc.sbuf_tensor("do_t", [P, RPP * D], f32))
        se_t = ctx.enter_context(nc.sbuf_tensor("se_t", [P, RPP], f32))
        so_t = ctx.enter_context(nc.sbuf_tensor("so_t", [P, RPP], f32))
        ds_t = ctx.enter_context(nc.sbuf_tensor("ds_t", [P, RPP], f32))
        s_t = ctx.enter_context(nc.sbuf_tensor("s_t", [P, RPP], f32))
        cl_t = ctx.enter_context(nc.sbuf_tensor("cl_t", [P, RPP], f32))
        s_w16 = ctx.enter_context(nc.semaphore("s_w16"))
        s_x = ctx.enter_context(nc.semaphore("s_x"))
        s_par = ctx.enter_context(nc.semaphore("s_par"))
        s_g = ctx.enter_context(nc.semaphore("s_g"))
        s_dve = ctx.enter_context(nc.semaphore("s_dve"))
        s_v = ctx.enter_context(nc.semaphore("s_v"))
        s_out = ctx.enter_context(nc.semaphore("s_out"))
        block = ctx.enter_context(nc.Block())

        @block.sync
        def _(sync: bass.BassEngine):
            # w16 first: it gates the gather
            sync.dma_start(out=w16_t[:], in_=w16_d[:]).then_inc(s_w16, 16)
            # x rows: x_t[p, c*D:(c+1)*D] = x[c*128 + p] (gather row layout)
            sync.dma_start(
                out=x_t[:].rearrange("p (n m) -> p n m", m=D),
                in_=x_d[:].rearrange("(n p) m -> p n m", p=P),
            ).then_inc(s_x, 16)
            # parity source: par_i[p, c] = labels[c*128 + p]; element-strided
            # (1024 4B descriptors) but fully shadowed by the gather
            with nc.allow_non_contiguous_dma(reason="4KB, hidden under gather"):
                sync.dma_start(
                    out=par_i[:], in_=lab_d[:].rearrange("(n p) -> p n", p=P)
                ).then_inc(s_par, 16)
            # writeback; no completion wait -- NEFF epilogue drains HWDGE
            sync.wait_ge(s_dve, 1)
            sync.dma_start(out=out_d[:], in_=cl_t[:]).then_inc(s_out, 16)

        @block.gpsimd
        def _(gpsimd: bass.BassGpSimd):
            # ucode swap runs while the input DMAs + index prep land
            gpsimd.load_library(mlp)
            gpsimd.wait_ge(s_v, 2)  # idx_t ready (shift + cast on DVE)
            gpsimd.dma_gather(
                pairs_t[:].rearrange("p (n m) -> p n m", m=2 * D),
                cen_d[:].rearrange("(q t) m -> q (t m)", t=2),
                idx_t[:],
                ROWS,
                ROWS,
                2 * D,
            ).then_inc(s_g, 16)

        @block.vector
        def _(vector: bass.BassEngine):
            # DVE has no same-engine interlock: s_v counts completions
            vector.wait_ge(s_w16, 16)
            vector.tensor_scalar(
                out=shr_t[:],
                in0=w16_t[:],
                scalar1=1,
                scalar2=None,
                op0=mybir.AluOpType.arith_shift_right,
            ).then_inc(s_v, 1)
            vector.wait_ge(s_v, 1)
            vector.tensor_copy(out=idx_t[:], in_=shr_t[:]).then_inc(s_v, 1)
            vector.wait_ge(s_par, 16)
            vector.tensor_scalar(
                out=parb_i[:],
                in0=par_i[:],
                scalar1=1,
                scalar2=None,
                op0=mybir.AluOpType.bitwise_and,
            ).then_inc(s_v, 1)
            vector.wait_ge(s_v, 3)
            vector.tensor_copy(out=par_f[:], in_=parb_i[:]).then_inc(s_v, 1)

            x_v = x_t[:].rearrange("p (n m) -> p n m", m=D)
            pairs_v = pairs_t[:].rearrange("p (n m) -> p n m", m=2 * D)
            vector.wait_ge(s_x, 16)
            vector.wait_ge(s_g, 16)
            vector.tensor_tensor(
                out=de_t[:].rearrange("p (n m) -> p n m", m=D),
                in0=x_v,
                in1=pairs_v[:, :, 0:D],
                op=mybir.AluOpType.subtract,
            ).then_inc(s_v, 1)  # 5
            vector.tensor_tensor(
                out=do_t[:].rearrange("p (n m) -> p n m", m=D),
                in0=x_v,
                in1=pairs_v[:, :, D : 2 * D],
                op=mybir.AluOpType.subtract,
            ).then_inc(s_v, 1)  # 6
            vector.wait_ge(s_v, 5)
            vector.tensor_tensor(
                out=de_t[:], in0=de_t[:], in1=de_t[:], op=mybir.AluOpType.mult
            ).then_inc(s_v, 1)  # 7
            vector.wait_ge(s_v, 6)
            vector.tensor_tensor(
                out=do_t[:], in0=do_t[:], in1=do_t[:], op=mybir.AluOpType.mult
            ).then_inc(s_v, 1)  # 8
            vector.wait_ge(s_v, 7)
            vector.reduce_sum(
                out=se_t[:],
                in_=de_t[:].rearrange("p (n m) -> p n m", m=D),
                axis=mybir.AxisListType.X,
            ).then_inc(s_v, 1)  # 9
            vector.wait_ge(s_v, 8)
            vector.reduce_sum(
                out=so_t[:],
                in_=do_t[:].rearrange("p (n m) -> p n m", m=D),
                axis=mybir.AxisListType.X,
            ).then_inc(s_v, 1)  # 10
            # s = s_even + par * (s_odd - s_even), then clamp
            vector.wait_ge(s_v, 10)
            vector.tensor_tensor(
                out=ds_t[:], in0=so_t[:], in1=se_t[:], op=mybir.AluOpType.subtract
            ).then_inc(s_v, 1)  # 11
            vector.wait_ge(s_v, 11)
            vector.tensor_tensor(
                out=ds_t[:], in0=ds_t[:], in1=par_f[:], op=mybir.AluOpType.mult
            ).then_inc(s_v, 1)  # 12
            vector.wait_ge(s_v, 12)
            vector.tensor_tensor(
                out=s_t[:], in0=se_t[:], in1=ds_t[:], op=mybir.AluOpType.add
            ).then_inc(s_v, 1)  # 13
            vector.wait_ge(s_v, 13)
            vector.tensor_scalar(
                out=cl_t[:],
                in0=s_t[:],
                scalar1=CLAMP_MIN,
                scalar2=CLAMP_MAX,
                op0=mybir.AluOpType.max,
                op1=mybir.AluOpType.min,
            ).then_inc(s_dve, 1)

    nc.compile()
    return nc


def _get_program():
    if "nc" not in _CACHE:
        _CACHE["nc"] = _build_program()
    return _CACHE["nc"]


def _wrap16(labels_i32):
    # labels_w16[p, s] = labels[s*16 + p%16], replicated to 128 partitions
    base = labels_i32.reshape(ROWS // 16, 16).T  # [16, ROWS//16]
    return np.ascontiguousarray(np.tile(base, (P // 16, 1)))


def kernel(x, labels, centers, trace=False):
    from concourse.bass_utils import run_bass_kernel_spmd

    nc = _get_program()

    x = np.ascontiguousarray(np.asarray(x, dtype=np.float32))
    labels_i32 = np.ascontiguousarray(np.asarray(labels, dtype=np.int32))
    centers = np.ascontiguousarray(np.asarray(centers, dtype=np.float32))

    in_maps = []
    for i in range(N_CORES):
        lab_c = labels_i32[i * ROWS : (i + 1) * ROWS]
        in_maps.append(
            {
                "x": x[i * ROWS : (i + 1) * ROWS],
                "labels": lab_c,
                "labels_w16": _wrap16(lab_c),
                "centers": centers,
            }
        )

    res = run_bass_kernel_spmd(
        nc, in_maps, core_ids=list(range(N_CORES)), trace=trace
    )

    total = np.float64(0.0)
    for r in res.results:
        total += np.sum(r["out"], dtype=np.float64)
    # masked-out entries: BS*(C_OUT-1) zeros, each clamped to 1e-12
    total += np.float64(BS) * np.float64(C_OUT - 1) * 1e-12
    loss = np.float32(total / BS)

    if trace:
        _CACHE["last_exec_time_ns"] = res.exec_time_ns
        _CACHE["last_results"] = res
    return np.array(loss, dtype=np.float32)



# revision 2
# speedup vs baseline: 1.1368x; 1.1368x over previous
"""CenterLoss Trainium2 kernel.

loss = (sum_i clamp(||x_i - centers[labels_i]||^2, 1e-12, 1e12)
        + BS*(C_OUT-1)*1e-12) / BS

Masking the full [BS, C_OUT] distance matrix keeps exactly one distance
per row; the other BS*(C_OUT-1) clamped zeros are a host-side constant.

Data-parallel over batch across 8 NeuronCores, centers replicated.  Each
core gathers its 1024 label rows with dma_gather ucode instructions
(int16 idx limit => gather 512-byte center PAIRS at idx=label>>1, select
the even/odd half per row from the label parity on DVE).

The gather is split into PIECES so the pipeline overlaps: descriptor
generation of piece k+1 (Q7 ucode, ~8ns/row serial on the Pool engine)
hides the DMA transfer of piece k, and the DVE math of piece k hides
under gen/transfer of piece k+1.  Piece sizes shrink toward the end so
only a small transfer+math tail remains after the last descriptor gen.
The mlp ucode library swap (load_library) is issued at kernel start; its
~11us background load gates the first gather's ucode entry, so the idx
prep path (w16 DMA + DVE shift/cast, ready ~10.4us) is not critical.
Raw Bass blocks (no TileContext) avoid the Tile kernel-tail barrier.
Host sums partials in f64.

Host-side input prep is limited to sharding/replication and index-tensor
layout (int64->int32 and the 16-partition-wrapped x8-replicated index
layout the gather ucode requires); index arithmetic (>>1, &1) and all
data math run on device.
"""

import numpy as np

BS, C_OUT, D = 8192, 50000, 64
N_CORES = 8
ROWS = BS // N_CORES  # rows per core
P = 128  # SBUF partitions
RPP = ROWS // P  # rows per partition (row r lives at [r % P, r // P])
CLAMP_MIN, CLAMP_MAX = 1e-12, 1e12
PIECES = (512, 384, 128)  # gather split; multiples of 128 summing to ROWS

_CACHE = {}


def _build_program():
    import concourse.bacc as bacc
    import concourse.bass as bass
    import concourse.mybir as mybir
    from concourse.library_config import mlp

    nc = bacc.Bacc(
        "TRN2", target_bir_lowering=False, debug=False, num_devices=N_CORES
    )

    f32 = mybir.dt.float32
    i32 = mybir.dt.int32
    i16 = mybir.dt.int16

    x_d = nc.dram_tensor("x", [ROWS, D], f32, kind="ExternalInput")
    lab_d = nc.dram_tensor("labels", [ROWS], i32, kind="ExternalInput")
    w16_d = nc.dram_tensor("labels_w16", [P, ROWS // 16], i32, kind="ExternalInput")
    cen_d = nc.dram_tensor("centers", [C_OUT, D], f32, kind="ExternalInput")
    out_d = nc.dram_tensor("out", [P, RPP], f32, kind="ExternalOutput")

    from contextlib import ExitStack
    with ExitStack() as ctx:
        x_t = ctx.enter_context(nc.sbuf_tensor("x_t", [P, RPP * D], f32))
        w16_t = ctx.enter_context(nc.sbuf_tensor("w16_t", [P, ROWS // 16], i32))
        shr_t = ctx.enter_context(nc.sbuf_tensor("shr_t", [P, ROWS // 16], i32))
        idx_t = ctx.enter_context(nc.sbuf_tensor("idx_t", [P, ROWS // 16], i16))
        par_i = ctx.enter_context(nc.sbuf_tensor("par_i", [P, RPP], i32))
        parb_i = ctx.enter_context(nc.sbuf_tensor("parb_i", [P, RPP], i32))
        par_f = ctx.enter_context(nc.sbuf_tensor("par_f", [P, RPP], f32))
        pairs_t = ctx.enter_context(nc.sbuf_tensor("pairs_t", [P, RPP * 2 * D], f32))
        de_t = ctx.enter_context(nc.sbuf_tensor("de_t", [P, RPP * D], f32))
        do_t = ctx.enter_context(nc.sbuf_tensor("do_t", [P, RPP * D], f32))
        se_t = ctx.enter_context(nc.sbuf_tensor("se_t", [P, RPP], f32))
        so_t = ctx.enter_context(nc.sbuf_tensor("so_t", [P, RPP], f32))
        ds_t = ctx.enter_context(nc.sbuf_tensor("ds_t", [P, RPP], f32))
        s_t = ctx.enter_context(nc.sbuf_tensor("s_t", [P, RPP], f32))
        cl_t = ctx.enter_context(nc.sbuf_tensor("cl_t", [P, RPP], f32))
        s_w16 = ctx.enter_context(nc.semaphore("s_w16"))
        s_x = ctx.enter_context(nc.semaphore("s_x"))
        s_par = ctx.enter_context(nc.semaphore("s_par"))
        s_g = [ctx.enter_context(nc.semaphore(f"s_g{i}")) for i in range(len(PIECES))]
        s_dve = ctx.enter_context(nc.semaphore("s_dve"))
        s_v = ctx.enter_context(nc.semaphore("s_v"))
        s_out = ctx.enter_context(nc.semaphore("s_out"))
        block = ctx.enter_context(nc.Block())

        @block.sync
        def _(sync: bass.BassEngine):
            # w16 first: it gates the gather idx prep
            sync.dma_start(out=w16_t[:], in_=w16_d[:]).then_inc(s_w16, 16)
            # x rows: x_t[p, c*D:(c+1)*D] = x[c*128 + p] (gather row layout)
            sync.dma_start(
                out=x_t[:].rearrange("p (n m) -> p n m", m=D),
                in_=x_d[:].rearrange("(n p) m -> p n m", p=P),
            ).then_inc(s_x, 16)
            # parity source: par_i[p, c] = labels[c*128 + p]; element-strided
            # but fully shadowed by the library swap + gather
            with nc.allow_non_contiguous_dma(reason="4KB, hidden under gather"):
                sync.dma_start(
                    out=par_i[:], in_=lab_d[:].rearrange("(n p) -> p n", p=P)
                ).then_inc(s_par, 16)
            # writeback; no completion wait -- NEFF epilogue drains HWDGE
            sync.wait_ge(s_dve, 1)
            sync.dma_start(out=out_d[:], in_=cl_t[:]).then_inc(s_out, 16)

        @block.gpsimd
        def _(gpsimd: bass.BassGpSimd):
            # ucode swap starts here; ~11us background load gates the first
            # gather's ucode entry (the real critical path start)
            gpsimd.load_library(mlp)
            gpsimd.wait_ge(s_v, 2)  # idx_t ready (shift + cast on DVE)
            r0 = 0
            for q, n in enumerate(PIECES):
                gpsimd.dma_gather(
                    pairs_t[:].rearrange("p (n m) -> p n m", m=2 * D)[
                        :, r0 // 128 : (r0 + n) // 128, :
                    ],
                    cen_d[:].rearrange("(q t) m -> q (t m)", t=2),
                    idx_t[:, r0 // 16 : (r0 + n) // 16],
                    n,
                    n,
                    2 * D,
                ).then_inc(s_g[q], 16)
                r0 += n

        @block.vector
        def _(vector: bass.BassEngine):
            # DVE has no same-engine interlock: s_v counts completions
            nv = 0
            vector.wait_ge(s_w16, 16)
            vector.tensor_scalar(
                out=shr_t[:],
                in0=w16_t[:],
                scalar1=1,
                scalar2=None,
                op0=mybir.AluOpType.arith_shift_right,
            ).then_inc(s_v, 1)
            nv += 1
            vector.wait_ge(s_v, nv)
            vector.tensor_copy(out=idx_t[:], in_=shr_t[:]).then_inc(s_v, 1)
            nv += 1  # == 2: gather may start
            # parity -> f32; runs during the library swap window
            vector.wait_ge(s_par, 16)
            vector.tensor_scalar(
                out=parb_i[:],
                in0=par_i[:],
                scalar1=1,
                scalar2=None,
                op0=mybir.AluOpType.bitwise_and,
            ).then_inc(s_v, 1)
            nv += 1
            vector.wait_ge(s_v, nv)
            vector.tensor_copy(out=par_f[:], in_=parb_i[:]).then_inc(s_v, 1)
            nv += 1

            x_v = x_t[:].rearrange("p (n m) -> p n m", m=D)
            pairs_v = pairs_t[:].rearrange("p (n m) -> p n m", m=2 * D)
            de_v = de_t[:].rearrange("p (n m) -> p n m", m=D)
            do_v = do_t[:].rearrange("p (n m) -> p n m", m=D)

            vector.wait_ge(s_x, 16)
            r0 = 0
            for q, n in enumerate(PIECES):
                sl = slice(r0 // 128, (r0 + n) // 128)
                fl = slice(r0 // 128 * D, (r0 + n) // 128 * D)
                vector.wait_ge(s_g[q], 16)
                vector.tensor_tensor(
                    out=de_v[:, sl, :],
                    in0=x_v[:, sl, :],
                    in1=pairs_v[:, sl, 0:D],
                    op=mybir.AluOpType.subtract,
                ).then_inc(s_v, 1)
                nv += 1
                vector.tensor_tensor(
                    out=do_v[:, sl, :],
                    in0=x_v[:, sl, :],
                    in1=pairs_v[:, sl, D : 2 * D],
                    op=mybir.AluOpType.subtract,
                ).then_inc(s_v, 1)
                nv += 1
                vector.wait_ge(s_v, nv - 1)
                vector.tensor_tensor(
                    out=de_t[:, fl], in0=de_t[:, fl], in1=de_t[:, fl],
                    op=mybir.AluOpType.mult,
                ).then_inc(s_v, 1)
                nv += 1
                vector.wait_ge(s_v, nv - 1)
                vector.tensor_tensor(
                    out=do_t[:, fl], in0=do_t[:, fl], in1=do_t[:, fl],
                    op=mybir.AluOpType.mult,
                ).then_inc(s_v, 1)
                nv += 1
                vector.wait_ge(s_v, nv - 1)
                vector.reduce_sum(
                    out=se_t[:, sl], in_=de_v[:, sl, :], axis=mybir.AxisListType.X
                ).then_inc(s_v, 1)
                nv += 1
                vector.wait_ge(s_v, nv - 1)
                vector.reduce_sum(
                    out=so_t[:, sl], in_=do_v[:, sl, :], axis=mybir.AxisListType.X
                ).then_inc(s_v, 1)
                nv += 1
                r0 += n

            # s = s_even + par * (s_odd - s_even), then clamp
            vector.wait_ge(s_v, nv)
            vector.tensor_tensor(
                out=ds_t[:], in0=so_t[:], in1=se_t[:], op=mybir.AluOpType.subtract
            ).then_inc(s_v, 1)
            nv += 1
            vector.wait_ge(s_v, nv)
            vector.tensor_tensor(
                out=ds_t[:], in0=ds_t[:], in1=par_f[:], op=mybir.AluOpType.mult
            ).then_inc(s_v, 1)
            nv += 1
            vector.wait_ge(s_v, nv)
            vector.tensor_tensor(
                out=s_t[:], in0=se_t[:], in1=ds_t[:], op=mybir.AluOpType.add
            ).then_inc(s_v, 1)
            nv += 1
            vector.wait_ge(s_v, nv)
            vector.tensor_scalar(
                out=cl_t[:],
                in0=s_t[:],
                scalar1=CLAMP_MIN,
                scalar2=CLAMP_MAX,
                op0=mybir.AluOpType.max,
                op1=mybir.AluOpType.min,
            ).then_inc(s_dve, 1)

    nc.compile()
    return nc


def _get_program():
    if "nc" not in _CACHE:
        _CACHE["nc"] = _build_program()
    return _CACHE["nc"]


def _wrap16(labels_i32):
    # labels_w16[p, s] = labels[s*16 + p%16], replicated to 128 partitions
    base = labels_i32.reshape(ROWS // 16, 16).T  # [16, ROWS//16]
    return np.ascontiguousarray(np.tile(base, (P // 16, 1)))


def kernel(x, labels, centers, trace=False):
    from concourse.bass_utils import run_bass_kernel_spmd

    nc = _get_program()

    x = np.ascontiguousarray(np.asarray(x, dtype=np.float32))
    labels_i32 = np.ascontiguousarray(np.asarray(labels, dtype=np.int32))
    centers = np.ascontiguousarray(np.asarray(centers, dtype=np.float32))

    in_maps = []
    for i in range(N_CORES):
        lab_c = labels_i32[i * ROWS : (i + 1) * ROWS]
        in_maps.append(
            {
                "x": x[i * ROWS : (i + 1) * ROWS],
                "labels": lab_c,
                "labels_w16": _wrap16(lab_c),
                "centers": centers,
            }
        )

    res = run_bass_kernel_spmd(
        nc, in_maps, core_ids=list(range(N_CORES)), trace=trace
    )

    total = np.float64(0.0)
    for r in res.results:
        total += np.sum(r["out"], dtype=np.float64)
    # masked-out entries: BS*(C_OUT-1) zeros, each clamped to 1e-12
    total += np.float64(BS) * np.float64(C_OUT - 1) * 1e-12
    loss = np.float32(total / BS)

    if trace:
        _CACHE["last_exec_time_ns"] = res.exec_time_ns
        _CACHE["last_results"] = res
    return np.array(loss, dtype=np.float32)
